# revision 16
# baseline (speedup 1.0000x reference)
"""BiLSTM-CRF forward NLL on 8 Trainium2 NeuronCores (Bass/Tile).

kernel(**inputs) takes the full unsharded inputs (as produced by the
reference setup_inputs) and returns the full output (scalar f32 NLL sum).

Sharding: data-parallel over batch (64 seqs -> 8 seqs/core); parameters
replicated. Per core:
  A. embedding gather (indirect DMA) + PE transpose -> xT [E_part, st] bf16
  B. xs = W_ih @ x + b as bf16 GEMMs (gate rows permuted to [i,f,o,g]);
     bwd xs overwritten with -40 past seq end (gates shut -> state stays 0,
     replacing per-step masking); xs staged to HBM
  C. LSTM scan, fwd+bwd as two interleaved chains; gates-on-partition
  D. layer-1 xs from hs0, second scan
  E. emissions GEMM straight into CRF tree layout
  F. CRF logZ via exp-space tree reduction over [8,8] transition matrices;
     gold score via host-built one-hot masks; out = sum(logZ - gold)
"""

import numpy as np
import ml_dtypes

import concourse.bacc as bacc
import concourse.bass as bass
import concourse.mybir as mybir
from concourse.tile import TileContext
from concourse.bass_utils import run_bass_kernel_spmd

F32 = mybir.dt.float32
BF16 = mybir.dt.bfloat16
I32 = mybir.dt.int32
F8 = mybir.dt.float8e3
NPF8 = ml_dtypes.float8_e3m4
AF = mybir.ActivationFunctionType
ALU = mybir.AluOpType

V, E, H, LBL, LAYERS = 50000, 512, 256, 8, 2
B = 64
NCORES = 8
BS = B // NCORES          # seqs per core
T = 1024                  # overridable for dev via build(T=...)
NBIG = -1.0e9
XPEN = -40.0
QP = 64                   # CRF matrices per partition

# gate order [f, i, g, o]: f/i in the lo sigmoid half (t2 starts early),
# g doubled for the sigmoid-tanh trick, o only needed late (h)
GPERM = np.concatenate([
    np.arange(256, 512), np.arange(0, 256),
    np.arange(512, 768), np.arange(768, 1024)])


# ============================================================================
# host-side preparation
# ============================================================================

def prep_shared(emb, W_ih, W_hh, b_lstm, W_out, b_out, trans, start_t, end_t):
    d = {}
    d["emb"] = np.ascontiguousarray(emb, dtype=np.float32)
    wih = np.empty((LAYERS, 2, 4, 8, 128, 128), dtype=np.float32)
    whh = np.empty((LAYERS, 2, 2, 8, 128, 128), dtype=np.float32)
    biases = np.empty((LAYERS, 2, 128, 8), dtype=np.float32)
    # g-gate rows (permuted order: gates 512:768 = m-chunks 4,5) are doubled
    # so a single sigmoid computes sigmoid(2x); tanh(x) = 2*sigmoid(2x)-1 via
    # a one-op DVE fixup.
    gdbl = np.ones((1024, 1), np.float32)
    gdbl[512:768] = 2.0
    for l in range(LAYERS):
        for dd in range(2):
            lt = (W_ih[l, dd][GPERM] * gdbl).T.astype(np.float32)  # [512,1024]
            wih[l, dd] = lt.reshape(4, 128, 8, 128).transpose(0, 2, 1, 3)
            lth = (W_hh[l, dd][GPERM] * gdbl).T.astype(np.float32)  # [256,1024]
            whh[l, dd] = lth.reshape(2, 128, 8, 128).transpose(0, 2, 1, 3)
            biases[l, dd] = (b_lstm[l, dd][GPERM] * gdbl[:, 0]).reshape(8, 128).T
    d["wih0"] = wih[0].astype(ml_dtypes.bfloat16)
    d["wih1"] = wih[1].astype(NPF8)
    d["whh"] = whh.astype(NPF8)
    d["biasv"] = biases
    d["woutT"] = np.ascontiguousarray(
        W_out.T.astype(np.float32).reshape(4, 128, 8)).astype(NPF8)
    d["bout_rep"] = np.broadcast_to(
        b_out.astype(np.float32)[None, :], (128, 8)).copy()
    d["identity"] = np.eye(128, dtype=np.float32)
    d["ntransflat"] = np.ascontiguousarray(
        -trans.astype(np.float32).reshape(64, 1))
    d["startv"] = start_t.astype(np.float32).reshape(8, 1).copy()
    d["endv"] = end_t.astype(np.float32).reshape(8, 1).copy()
    d["end_rep"] = np.broadcast_to(
        end_t.astype(np.float32)[None, :], (8, 8)).copy()
    d["ones8"] = np.ones((8, 1), dtype=np.float32)
    d["nones128"] = -np.ones((128, 1), dtype=np.float32)
    return d


def prep_core(core, tokens, tags, lengths, trans, start_t):
    ST = BS * T
    NP = ST // QP
    s0 = core * BS
    tok = tokens[s0:s0 + BS].astype(np.int64)
    tg = tags[s0:s0 + BS].astype(np.int64)
    ln = np.maximum(lengths[s0:s0 + BS].astype(np.int64), 1)

    st = np.arange(ST)
    st_u = st // T
    st_t = st % T

    d = {}
    d["tokidx"] = np.ascontiguousarray(
        tok[st_u, st_t].reshape(ST // 128, 128).T).astype(np.int32)

    mask_st = (st_t < ln[st_u]).astype(np.float32)
    d["xmask_rep"] = np.broadcast_to(
        mask_st[None, :], (128, ST)).astype(ml_dtypes.bfloat16)
    d["xpen_rep"] = np.broadcast_to(
        (XPEN * (1.0 - mask_st))[None, :], (128, ST)).astype(ml_dtypes.bfloat16)

    valid = mask_st > 0
    ohe = np.zeros((NP, QP * 8), dtype=np.float32)
    ohe[(st // QP)[valid], (st % QP)[valid] * 8 + tg[st_u[valid], st_t[valid]]] = 1.0
    d["ohe"] = ohe.astype(ml_dtypes.bfloat16)

    oh2 = np.zeros((64, ST), dtype=np.float32)
    v2 = (st_t >= 1) & (st_t < ln[st_u])
    pair = tg[st_u, st_t] * 8 + tg[st_u, np.maximum(st_t - 1, 0)]
    oh2[pair[v2], st[v2]] = 1.0
    d["oh2T"] = oh2.astype(ml_dtypes.bfloat16)

    cnt0 = np.zeros(8, dtype=np.float32)
    cntE = np.zeros(8, dtype=np.float32)
    for u in range(BS):
        cnt0[tg[u, 0]] += 1.0
        cntE[tg[u, ln[u] - 1]] += 1.0
    d["ncnt0"] = np.ascontiguousarray(-cnt0.reshape(8, 1))
    d["ncntE"] = np.ascontiguousarray(-cntE.reshape(8, 1))

    em = np.zeros((NP, QP), dtype=np.float32)
    em[st // QP, st % QP] = mask_st
    d["em"] = em

    trans32 = trans.astype(np.float32)
    start_mat = np.broadcast_to(start_t.astype(np.float32)[:, None], (8, 8))
    ilog = np.full((8, 8), NBIG, dtype=np.float32)
    np.fill_diagonal(ilog, 0.0)
    leaf = np.where(st_t[:, None, None] == 0, start_mat[None],
                    np.where((st_t < ln[st_u])[:, None, None], trans32[None],
                             ilog[None]))
    tmx = np.zeros((NP, QP * 64), dtype=np.float32)
    tmx[(st // QP)[:, None], ((st % QP) * 64)[:, None] + np.arange(64)[None, :]] = \
        leaf.reshape(ST, 64)
    d["tmx"] = tmx
    return d


# ============================================================================
# device program
# ============================================================================

def build_program(debug_taps=(), t_override=None):
    global T
    if t_override is not None:
        T = t_override
    ST = BS * T
    NP = ST // QP            # CRF partitions in use
    NC128 = ST // 128        # gather calls
    NSL = ST // 512          # 512-wide GEMM slices
    CH = min(64, T)          # scan prefetch chunk
    NCH = T // CH
    XLV = 6                  # in-partition tree levels (64 -> 1)
    CLV = int(np.log2(max(NP // BS, 1)))   # cross-partition levels

    nc = bacc.Bacc("TRN2", target_bir_lowering=False, debug=False,
                   num_devices=NCORES)
    dp = nc.declare_dram_parameter
    P = {}
    P["emb"] = dp("emb", [V, E], F32, isOutput=False)
    P["wih0"] = dp("wih0", [2, 4, 8, 128, 128], BF16, isOutput=False)
    P["wih1"] = dp("wih1", [2, 4, 8, 128, 128], F8, isOutput=False)
    P["whh"] = dp("whh", [LAYERS, 2, 2, 8, 128, 128], F8, isOutput=False)
    P["biasv"] = dp("biasv", [LAYERS, 2, 128, 8], F32, isOutput=False)
    P["woutT"] = dp("woutT", [4, 128, 8], F8, isOutput=False)
    P["bout_rep"] = dp("bout_rep", [128, 8], F32, isOutput=False)
    P["identity"] = dp("identity", [128, 128], F32, isOutput=False)
    P["ntransflat"] = dp("ntransflat", [64, 1], F32, isOutput=False)
    P["startv"] = dp("startv", [8, 1], F32, isOutput=False)
    P["endv"] = dp("endv", [8, 1], F32, isOutput=False)
    P["end_rep"] = dp("end_rep", [8, 8], F32, isOutput=False)
    P["ones8"] = dp("ones8", [8, 1], F32, isOutput=False)
    P["nones128"] = dp("nones128", [128, 1], F32, isOutput=False)
    P["tokidx"] = dp("tokidx", [128, NC128], I32, isOutput=False)
    P["xmask_rep"] = dp("xmask_rep", [128, ST], BF16, isOutput=False)
    P["xpen_rep"] = dp("xpen_rep", [128, ST], BF16, isOutput=False)
    P["ohe"] = dp("ohe", [NP, QP * 8], BF16, isOutput=False)
    P["oh2T"] = dp("oh2T", [64, ST], BF16, isOutput=False)
    P["ncnt0"] = dp("ncnt0", [8, 1], F32, isOutput=False)
    P["ncntE"] = dp("ncntE", [8, 1], F32, isOutput=False)
    P["em"] = dp("em", [NP, QP], F32, isOutput=False)
    P["tmx"] = dp("tmx", [NP, QP * 64], F32, isOutput=False)
    P["outv"] = dp("outv", [1, 1], F32, isOutput=True)
    xs_hbm = [nc.dram_tensor(f"xs_hbm{d}", [8, 128, ST], BF16)
              for d in range(2)]

    def dbg_dump(tc_nc, name, ap, shape, dtype):
        t = tc_nc.dram_tensor(name, shape, dtype, kind="ExternalOutput")
        tc_nc.sync.dma_start(out=t[:], in_=ap)

    with TileContext(nc) as tc:
        # ---- global constants ----
        with tc.tile_pool(name="gconst", bufs=1) as gc:
            wih_sb = {0: gc.tile([128, 2 * 4 * 8 * 128], BF16, tag="wih0", name="wih0sb"),
                      1: gc.tile([128, 2 * 4 * 8 * 128], F8, tag="wih1", name="wih1sb")}
            whh_sb = gc.tile([128, LAYERS * 2 * 2 * 8 * 128], F8, tag="whh")
            bias_sb = gc.tile([128, LAYERS * 2 * 8], F32, tag="bias")
            for l in range(LAYERS):
                for d in range(2):
                    o = d * 4 * 8 * 128
                    nc.sync.dma_start(
                        out=wih_sb[l][:, o:o + 4 * 8 * 128].rearrange(
                            "p (k m q) -> p k m q", k=4, m=8),
                        in_=P[f"wih{l}"][:][d].rearrange("k m p q -> p k m q"))
                    o = (l * 2 + d) * 2 * 8 * 128
                    nc.sync.dma_start(
                        out=whh_sb[:, o:o + 2 * 8 * 128].rearrange(
                            "p (k m q) -> p k m q", k=2, m=8),
                        in_=P["whh"][:][l, d].rearrange("k m p q -> p k m q"))
                    o = (l * 2 + d) * 8
                    nc.sync.dma_start(out=bias_sb[:, o:o + 8],
                                      in_=P["biasv"][:][l, d])
            ident_sb = gc.tile([128, 128], F32, tag="ident")
            nc.sync.dma_start(out=ident_sb[:], in_=P["identity"][:])
            identb_sb = gc.tile([128, 128], BF16, tag="identb")
            nc.vector.tensor_copy(out=identb_sb[:], in_=ident_sb[:])
            xmask_sb = gc.tile([128, ST], BF16, tag="xmask")
            nc.sync.dma_start(out=xmask_sb[:], in_=P["xmask_rep"][:])
            xpen_sb = gc.tile([128, ST], BF16, tag="xpen")
            nc.sync.dma_start(out=xpen_sb[:], in_=P["xpen_rep"][:])

            def wih_t(l, d, kc, m):
                i = (d * 4 + kc) * 8 + m
                return wih_sb[l][:, i * 128:(i + 1) * 128]

            def whh_t(l, d, kc, m):
                i = ((l * 2 + d) * 2 + kc) * 8 + m
                return whh_sb[:, i * 128:(i + 1) * 128]

            def bias_col(l, d, m):
                i = (l * 2 + d) * 8 + m
                return bias_sb[:, i:i + 1]

            # ---- input-projection GEMM (shared by both layers) ----
            def xproj_slice(l, rhs_chunk, sp, pp, d, m, s):
                ps = pp.tile([128, 512], F32, tag="g")
                for kc in range(4):
                    nc.tensor.matmul(
                        ps[:], lhsT=wih_t(l, d, kc, m),
                        rhs=rhs_chunk(kc)[:, s * 512:(s + 1) * 512],
                        start=(kc == 0), stop=(kc == 3))
                stg = sp.tile([128, 512], BF16, tag="xstage")
                if d == 0:
                    nc.vector.tensor_scalar_add(
                        stg[:], ps[:], bias_col(l, d, m))
                else:
                    nc.vector.scalar_tensor_tensor(
                        out=stg[:], in0=ps[:],
                        scalar=bias_col(l, d, m),
                        in1=xmask_sb[:, s * 512:(s + 1) * 512],
                        op0=ALU.add, op1=ALU.mult)
                    nc.vector.tensor_tensor(
                        out=stg[:], in0=stg[:],
                        in1=xpen_sb[:, s * 512:(s + 1) * 512],
                        op=ALU.add)
                nc.sync.dma_start(
                    out=xs_hbm[d][:][m, :, s * 512:(s + 1) * 512],
                    in_=stg[:])

            def xproj_layer(l, rhs_chunk):
                with tc.tile_pool(name=f"xp{l}", bufs=4) as sp, \
                     tc.tile_pool(name=f"xpp{l}", bufs=2, space="PSUM") as pp:
                    for d in range(2):
                        for m in range(8):
                            for s in range(NSL):
                                xproj_slice(l, rhs_chunk, sp, pp, d, m, s)

            # ---- LSTM scan (both dirs interleaved) ----
            def scan_layer(l, hs):
                with tc.tile_pool(name=f"sc{l}", bufs=4) as sp, \
                     tc.tile_pool(name=f"scs{l}", bufs=2) as strm, \
                     tc.tile_pool(name=f"scst{l}", bufs=1) as stp, \
                     tc.tile_pool(name=f"scp{l}", bufs=2, space="PSUM") as pp:
                    c_st = {d: stp.tile([128, 16], F32, tag=f"c{d}", name=f"c{d}")
                            for d in range(2)}
                    for d in range(2):
                        nc.vector.memset(c_st[d][:], 0.0)

                    def fetch(d, k):
                        buf = strm.tile([128, 64 * CH], BF16, tag=f"xsb{d}", name=f"xsb{d}")
                        t0 = k * CH
                        for m in range(8):
                            nc.sync.dma_start(
                                out=buf[:, m * BS * CH:(m + 1) * BS * CH]
                                .rearrange("p (u ch) -> p u ch", u=BS),
                                in_=xs_hbm[d][:][m].rearrange(
                                    "p (u t) -> p u t", u=BS)[:, :, t0:t0 + CH])
                        # relayout to t-major on the otherwise-idle gpsimd so
                        # the per-step identity matmuls get a contiguous rhs
                        tb = strm.tile([128, 64 * CH], BF16, tag=f"xst{d}",
                                       name=f"xst{d}")
                        nc.gpsimd.tensor_copy(
                            out=tb[:].rearrange("p (ch m u) -> p ch m u",
                                                ch=CH, m=8),
                            in_=buf[:].rearrange("p (m u ch) -> p ch m u",
                                                 m=8, u=BS))
                        return tb

                    def id_phase(d, t, tb):
                        # xs injection: identity matmul with contiguous rhs;
                        # issued one step ahead so it fills the PE idle gap
                        # while the pointwise chain runs. lo/hi are separate
                        # PSUM tiles = separate accumulation groups.
                        trel = t % CH
                        first = (t == 0) if d == 0 else (t == T - 1)
                        Glo = pp.tile([128, 32], F32, tag=f"Glo{d}")
                        Ghi = pp.tile([128, 32], F32, tag=f"Ghi{d}")
                        nc.tensor.matmul(Glo[:], lhsT=identb_sb[:],
                                         rhs=tb[:, trel * 64:trel * 64 + 32],
                                         start=True, stop=first)
                        nc.tensor.matmul(Ghi[:], lhsT=identb_sb[:],
                                         rhs=tb[:, trel * 64 + 32:trel * 64 + 64],
                                         start=True, stop=first)
                        return Glo, Ghi

                    def whh_phase(d, t, G2):
                        # gate order [f,i | g,o]: lo half finishes first so
                        # sigmoid-lo/t2 start before the hi MMs end; kc-major
                        # within each half so the burst head waits h-chunk0
                        # only
                        Glo, Ghi = G2
                        first = (t == 0) if d == 0 else (t == T - 1)
                        if first:
                            return
                        tprev = t - 1 if d == 0 else t + 1
                        hv = hs[d][:].rearrange(
                            "p (c u t) -> p c u t", c=2, u=BS)
                        for g0, Gt in ((0, Glo), (4, Ghi)):
                            for kc in range(2):
                                for m in range(g0, g0 + 4):
                                    mm = m - g0
                                    nc.tensor.matmul(
                                        Gt[:, mm * 8:(mm + 1) * 8],
                                        lhsT=whh_t(l, d, kc, m),
                                        rhs=hv[:, kc, :, tprev],
                                        start=False,
                                        stop=(kc == 1 and m == g0 + 3))

                    def sig_lo(d, Glo):
                        S = sp.tile([128, 64], BF16, tag=f"S{d}")
                        nc.scalar.activation(S[:, 0:32], Glo[:], AF.Sigmoid)
                        return S

                    def sig_hi(d, Ghi, S):
                        nc.scalar.activation(S[:, 32:64], Ghi[:], AF.Sigmoid)

                    def t2_phase(d, S):
                        # t2 = sig(f) * c_prev ; f-gates live in the lo half
                        t2 = sp.tile([128, 16], F32, tag=f"t2{d}")
                        nc.vector.tensor_tensor(out=t2[:], in0=S[:, 0:16],
                                                in1=c_st[d][:], op=ALU.mult)
                        return t2

                    def c_phase(d, S, t2):
                        # t1h = (sig(2g) - 0.5) * i  == 0.5 * i * tanh(g)
                        t1 = sp.tile([128, 16], F32, tag=f"t1{d}")
                        nc.vector.scalar_tensor_tensor(
                            out=t1[:], in0=S[:, 32:48], scalar=0.5,
                            in1=S[:, 16:32], op0=ALU.subtract, op1=ALU.mult)
                        # c = 2*t1h + t2
                        nc.vector.scalar_tensor_tensor(
                            out=c_st[d][:], in0=t1[:], scalar=2.0, in1=t2[:],
                            op0=ALU.mult, op1=ALU.add)

                    def tanh_phase(d):
                        Tc = sp.tile([128, 16], F32, tag=f"Tc{d}")
                        nc.scalar.activation(Tc[:], c_st[d][:], AF.Tanh)
                        return Tc

                    def h_phase(d, t, S, Tc):
                        # split by h-chunk so next step's kc0 matmuls gate on
                        # chunk0 only
                        hv = hs[d][:].rearrange(
                            "p (c u t) -> p c u t", c=2, u=BS)
                        nc.vector.tensor_tensor(out=hv[:, 0, :, t],
                                                in0=S[:, 48:56],
                                                in1=Tc[:, 0:8], op=ALU.mult)
                        nc.vector.tensor_tensor(out=hv[:, 1, :, t],
                                                in0=S[:, 56:64],
                                                in1=Tc[:, 8:16], op=ALU.mult)

                    bufs = {0: fetch(0, 0), 1: fetch(1, NCH - 1)}
                    for k in range(NCH):
                        nxt = None
                        if k + 1 < NCH:
                            nxt = (fetch(0, k + 1), fetch(1, NCH - 2 - k))
                        for i in range(CH):
                            tf = k * CH + i
                            tb = T - 1 - tf
                            # phase-staggered issue: each engine's FIFO sees
                            # the two chains' ops in dependency-friendly order
                            Gf = id_phase(0, tf, bufs[0])
                            whh_phase(0, tf, Gf)
                            Sf = sig_lo(0, Gf[0])
                            Gb = id_phase(1, tb, bufs[1])
                            whh_phase(1, tb, Gb)
                            t2f = t2_phase(0, Sf)
                            sig_hi(0, Gf[1], Sf)
                            Sb = sig_lo(1, Gb[0])
                            c_phase(0, Sf, t2f)
                            sig_hi(1, Gb[1], Sb)
                            t2b = t2_phase(1, Sb)
                            Tcf = tanh_phase(0)
                            c_phase(1, Sb, t2b)
                            Tcb = tanh_phase(1)
                            h_phase(0, tf, Sf, Tcf)
                            h_phase(1, tb, Sb, Tcb)
                        if nxt is not None:
                            bufs[0], bufs[1] = nxt

            # ================= pipeline =================
            with tc.tile_pool(name="xt", bufs=1) as xt_pool:
                xT = xt_pool.tile([128, 4 * ST], BF16, tag="xT")
                rhs0 = lambda kc: xT[:, kc * ST:(kc + 1) * ST]
                with tc.tile_pool(name="gat", bufs=4) as gp, \
                     tc.tile_pool(name="gatp", bufs=4, space="PSUM") as gpp, \
                     tc.tile_pool(name="tokp", bufs=1) as tkp, \
                     tc.tile_pool(name="xp0", bufs=4) as xsp, \
                     tc.tile_pool(name="xpp0", bufs=2, space="PSUM") as xpp:
                    tok_sb = tkp.tile([128, NC128], I32, tag="tok")
                    nc.sync.dma_start(out=tok_sb[:], in_=P["tokidx"][:])
                    for j in range(NC128):
                        g = gp.tile([128, E], F32, tag="xg")
                        nc.gpsimd.indirect_dma_start(
                            out=g[:], out_offset=None, in_=P["emb"][:],
                            in_offset=bass.IndirectOffsetOnAxis(
                                ap=tok_sb[:, j:j + 1], axis=0))
                        for c in range(4):
                            pst = gpp.tile([128, 128], F32, tag="tp")
                            nc.tensor.transpose(
                                out=pst[:], in_=g[:, c * 128:(c + 1) * 128],
                                identity=ident_sb[:])
                            nc.vector.tensor_copy(
                                out=xT[:, c * ST + j * 128:c * ST + (j + 1) * 128],
                                in_=pst[:])
                        # xproj0 slice s only needs xT column-blocks <= j,
                        # so overlap the layer-0 GEMM with the gather
                        if j % 4 == 3:
                            s = j // 4
                            for d in range(2):
                                for m in range(8):
                                    xproj_slice(0, rhs0, xsp, xpp, d, m, s)
                if "xT" in debug_taps:
                    dbg_dump(nc, "dbg_xT", xT[:], [128, 4 * ST], BF16)

            with tc.tile_pool(name="hs0", bufs=1) as hs0_pool:
                hs0 = {d: hs0_pool.tile([128, 2 * ST], F8, tag=f"h{d}", name=f"hs0{d}")
                       for d in range(2)}
                scan_layer(0, hs0)
                if "hs0" in debug_taps:
                    for d in range(2):
                        dbg_dump(nc, f"dbg_hs0{d}", hs0[d][:], [128, 2 * ST], F8)
                xproj_layer(1, lambda kc: hs0[kc // 2][
                    :, (kc % 2) * ST:(kc % 2 + 1) * ST])

            with tc.tile_pool(name="earr", bufs=1) as ea_pool:
                e_arr = ea_pool.tile([NP, QP * 8], F32, tag="e_arr")
                with tc.tile_pool(name="hs1", bufs=1) as hs1_pool:
                    hs1 = {d: hs1_pool.tile([128, 2 * ST], F8, tag=f"h{d}", name=f"hs1{d}")
                           for d in range(2)}
                    scan_layer(1, hs1)
                    if "hs1" in debug_taps:
                        for d in range(2):
                            dbg_dump(nc, f"dbg_hs1{d}", hs1[d][:],
                                     [128, 2 * ST], F8)
                    # emissions into CRF layout
                    with tc.tile_pool(name="emc", bufs=1) as emc, \
                         tc.tile_pool(name="emp", bufs=2, space="PSUM") as epp:
                        woutT_sb = emc.tile([128, 4 * 8], F8, tag="wout")
                        nc.sync.dma_start(
                            out=woutT_sb[:].rearrange("p (k m) -> p k m", k=4),
                            in_=P["woutT"][:].rearrange("k p m -> p k m"))
                        bout_sb = emc.tile([128, 8], F32, tag="bout")
                        nc.sync.dma_start(out=bout_sb[:], in_=P["bout_rep"][:])
                        for q in range(QP):
                            ps = epp.tile([NP, 8], F32, tag="eps")
                            for kc in range(4):
                                d, c = divmod(kc, 2)
                                lh = hs1[d][:].rearrange(
                                    "p (c pp q) -> p c pp q", c=2, q=QP)[
                                    :, c, :, q]
                                nc.tensor.matmul(
                                    ps[:], lhsT=lh,
                                    rhs=woutT_sb[:, kc * 8:(kc + 1) * 8],
                                    start=(kc == 0), stop=(kc == 3))
                            nc.vector.tensor_tensor(
                                out=e_arr[:, q * 8:(q + 1) * 8], in0=ps[:],
                                in1=bout_sb[:NP, :], op=ALU.add)
                if "e_arr" in debug_taps:
                    dbg_dump(nc, "dbg_e_arr", e_arr[:], [NP, QP * 8], F32)

                # ---------------- CRF ----------------
                with tc.tile_pool(name="crfc", bufs=1) as cc, \
                     tc.tile_pool(name="crfw", bufs=1) as cw, \
                     tc.tile_pool(name="crfp", bufs=1, space="PSUM") as cpp:
                    em_sb = cc.tile([NP, QP], F32, tag="em")
                    nc.sync.dma_start(out=em_sb[:], in_=P["em"][:])
                    tmx_sb = cc.tile([NP, QP * 64], F32, tag="tmx")
                    nc.sync.dma_start(out=tmx_sb[:], in_=P["tmx"][:])
                    ohe_sb = cc.tile([NP, QP * 8], BF16, tag="ohe")
                    nc.sync.dma_start(out=ohe_sb[:], in_=P["ohe"][:])
                    oh2_sb = cc.tile([64, ST], BF16, tag="oh2")
                    nc.sync.dma_start(out=oh2_sb[:], in_=P["oh2T"][:])
                    ntr_sb = cc.tile([64, 1], F32, tag="ntr")
                    nc.sync.dma_start(out=ntr_sb[:], in_=P["ntransflat"][:])
                    sv_sb = cc.tile([8, 1], F32, tag="sv")
                    nc.sync.dma_start(out=sv_sb[:], in_=P["startv"][:])
                    ev_sb = cc.tile([8, 1], F32, tag="ev")
                    nc.sync.dma_start(out=ev_sb[:], in_=P["endv"][:])
                    erep_sb = cc.tile([8, 8], F32, tag="erep")
                    nc.sync.dma_start(out=erep_sb[:], in_=P["end_rep"][:])
                    on8_sb = cc.tile([8, 1], F32, tag="on8")
                    nc.sync.dma_start(out=on8_sb[:], in_=P["ones8"][:])
                    no128_sb = cc.tile([128, 1], F32, tag="no128")
                    nc.sync.dma_start(out=no128_sb[:], in_=P["nones128"][:])
                    nc0_sb = cc.tile([8, 1], F32, tag="nc0")
                    nc.sync.dma_start(out=nc0_sb[:], in_=P["ncnt0"][:])
                    ncE_sb = cc.tile([8, 1], F32, tag="ncE")
                    nc.sync.dma_start(out=ncE_sb[:], in_=P["ncntE"][:])

                    # leaves: M = e*em + tmx  (log space), then exp w/ max
                    X = cw.tile([NP, QP * 64], F32, tag="X")
                    e_b = e_arr[:].rearrange("p (q i) -> p q i", i=8) \
                        .unsqueeze(3).to_broadcast([NP, QP, 8, 8])
                    em_b = em_sb[:].unsqueeze(2).unsqueeze(3) \
                        .to_broadcast([NP, QP, 8, 8])
                    Xv = X[:].rearrange("p (q i j) -> p q i j", i=8, j=8)
                    nc.vector.tensor_tensor(out=Xv, in0=e_b, in1=em_b,
                                            op=ALU.mult)
                    nc.vector.tensor_tensor(out=X[:], in0=X[:], in1=tmx_sb[:],
                                            op=ALU.add)
                    Sc = cw.tile([NP, QP], F32, tag="Sc")
                    nc.vector.tensor_reduce(
                        out=Sc[:].unsqueeze(2),
                        in_=X[:].rearrange("p (q e) -> p q e", e=64),
                        axis=mybir.AxisListType.X, op=ALU.max)
                    scb = Sc[:].unsqueeze(2).to_broadcast([NP, QP, 64]) \
                        .rearrange("p q e -> p q e")
                    nc.vector.tensor_tensor(
                        out=X[:].rearrange("p (q e) -> p q e", e=64),
                        in0=X[:].rearrange("p (q e) -> p q e", e=64),
                        in1=scb, op=ALU.subtract)
                    nc.scalar.activation(X[:], X[:], AF.Exp)

                    tmpa = cw.tile([NP, (QP // 2) * 64], F32, tag="tmpa")
                    tmpb = cw.tile([NP, (QP // 2) * 64], F32, tag="tmpb")

                    def combine(Xa, Xb, Sa, Sb, Pn, r, Y, Sy):
                        """Y = Xa x Xb (exp-space matmul over k), renormed.
                        Xa/Xb views [Pn, r, 8, 8]; Sa/Sb [Pn, r]; Y [Pn, r*64]."""
                        Yv = Y[:Pn, :r * 64].rearrange(
                            "p (r i j) -> p r i j", i=8, j=8)
                        Tv = tmpa[:Pn, :r * 64].rearrange(
                            "p (r i j) -> p r i j", i=8, j=8)
                        for kk in range(8):
                            ak = Xa[:, :, :, kk].unsqueeze(3) \
                                .to_broadcast([Pn, r, 8, 8])
                            bk = Xb[:, :, kk, :].unsqueeze(2) \
                                .to_broadcast([Pn, r, 8, 8])
                            dst = Yv if kk == 0 else Tv
                            nc.vector.tensor_tensor(out=dst, in0=ak, in1=bk,
                                                    op=ALU.mult)
                            if kk > 0:
                                nc.vector.tensor_tensor(out=Yv, in0=Yv, in1=Tv,
                                                        op=ALU.add)
                        # renormalize
                        am = tmpb[:Pn, :r]
                        nc.vector.tensor_reduce(
                            out=am.unsqueeze(2),
                            in_=Y[:Pn, :r * 64].rearrange("p (r e) -> p r e",
                                                          e=64),
                            axis=mybir.AxisListType.X, op=ALU.max)
                        inv = tmpb[:Pn, r:2 * r]
                        nc.vector.reciprocal(out=inv, in_=am)
                        nc.vector.tensor_tensor(
                            out=Y[:Pn, :r * 64].rearrange("p (r e) -> p r e", e=64),
                            in0=Y[:Pn, :r * 64].rearrange("p (r e) -> p r e", e=64),
                            in1=inv.unsqueeze(2).to_broadcast([Pn, r, 64]),
                            op=ALU.mult)
                        lam = tmpb[:Pn, 2 * r:3 * r]
                        nc.scalar.activation(lam, am, AF.Ln)
                        nc.vector.tensor_tensor(out=Sy, in0=Sa, in1=Sb,
                                                op=ALU.add)
                        nc.vector.tensor_tensor(out=Sy, in0=Sy, in1=lam,
                                                op=ALU.add)

                    # in-partition levels
                    Y = cw.tile([NP, (QP // 2) * 64], F32, tag="Y")
                    S2 = cw.tile([NP, QP], F32, tag="S2")
                    curX, curS = X, Sc
                    dstX, dstS = Y, S2
                    n = QP
                    while n > 1:
                        r = n // 2
                        Xq = curX[:NP, :n * 64].rearrange(
                            "p (r two i j) -> p r two i j", two=2, i=8, j=8)
                        combine(Xq[:, :, 1], Xq[:, :, 0],
                                curS[:NP, 1:n:2], curS[:NP, 0:n:2],
                                NP, r, dstX, dstS[:NP, :r])
                        curX, dstX = dstX, curX
                        curS, dstS = dstS, curS
                        n = r
                    # cross-partition levels: Pn partitions -> Pn/2
                    Pn = NP
                    Za = cw.tile([max(NP // 2, 8), 64], F32, tag="Za")
                    Zb = cw.tile([max(NP // 2, 8), 64], F32, tag="Zb")
                    Zsa = cw.tile([max(NP // 2, 8), 1], F32, tag="Zsa")
                    Zsb = cw.tile([max(NP // 2, 8), 1], F32, tag="Zsb")
                    for _ in range(CLV):
                        h = Pn // 2
                        nc.sync.dma_start(out=Za[:h, :], in_=curX[1:Pn:2, :64])
                        nc.sync.dma_start(out=Zb[:h, :], in_=curX[0:Pn:2, :64])
                        nc.sync.dma_start(out=Zsa[:h, :], in_=curS[1:Pn:2, 0:1])
                        nc.sync.dma_start(out=Zsb[:h, :], in_=curS[0:Pn:2, 0:1])
                        combine(Za[:h, :].rearrange("p (o i j) -> p o i j",
                                                    o=1, i=8, j=8),
                                Zb[:h, :].rearrange("p (o i j) -> p o i j",
                                                    o=1, i=8, j=8),
                                Zsa[:h, :], Zsb[:h, :],
                                h, 1, dstX, dstS[:h, :1])
                        curX, dstX = dstX, curX
                        curS, dstS = dstS, curS
                        Pn = h
                    # root: curX [BS, 64] exp-space, curS [BS, 1]
                    eend = cw.tile([8, 8], F32, tag="eend")
                    nc.scalar.activation(eend[:], erep_sb[:], AF.Exp)
                    w = cw.tile([8, 8], F32, tag="w")
                    nc.vector.tensor_tensor(out=w[:], in0=curX[:BS, 0:64:8],
                                            in1=eend[:], op=ALU.mult)
                    zs = cw.tile([8, 1], F32, tag="zs")
                    nc.vector.tensor_reduce(out=zs[:], in_=w[:],
                                            axis=mybir.AxisListType.X,
                                            op=ALU.add)
                    nc.scalar.activation(zs[:], zs[:], AF.Ln)
                    logz = cw.tile([8, 1], F32, tag="logz")
                    nc.vector.tensor_tensor(out=logz[:], in0=zs[:],
                                            in1=curS[:BS, 0:1], op=ALU.add)
                    if "logz" in debug_taps:
                        dbg_dump(nc, "dbg_logz", logz[:], [8, 1], F32)

                    # ---- gold + final sum into one psum scalar ----
                    acc = cpp.tile([1, 1], F32, tag="acc")
                    nc.tensor.matmul(acc[:], lhsT=on8_sb[:], rhs=logz[:],
                                     start=True, stop=False)
                    # emission gold term (negated)
                    prod = cw.tile([NP, QP * 8], F32, tag="prod")
                    nc.vector.tensor_tensor(out=prod[:], in0=e_arr[:],
                                            in1=ohe_sb[:], op=ALU.mult)
                    red = cw.tile([NP, 1], F32, tag="red")
                    nc.vector.tensor_reduce(out=red[:], in_=prod[:],
                                            axis=mybir.AxisListType.X,
                                            op=ALU.add)
                    nc.tensor.matmul(acc[:], lhsT=no128_sb[:NP, :], rhs=red[:],
                                     start=False, stop=False)
                    # transition gold term (negated)
                    rs2 = cw.tile([64, 1], F32, tag="rs2")
                    nc.vector.tensor_reduce(out=rs2[:], in_=oh2_sb[:],
                                            axis=mybir.AxisListType.X,
                                            op=ALU.add)
                    nc.tensor.matmul(acc[:], lhsT=ntr_sb[:], rhs=rs2[:],
                                     start=False, stop=False)
                    # start/end gold terms (negated counts)
                    nc.tensor.matmul(acc[:], lhsT=nc0_sb[:], rhs=sv_sb[:],
                                     start=False, stop=False)
                    nc.tensor.matmul(acc[:], lhsT=ncE_sb[:], rhs=ev_sb[:],
                                     start=False, stop=True)
                    res = cw.tile([1, 1], F32, tag="res")
                    nc.vector.tensor_copy(out=res[:], in_=acc[:])
                    nc.sync.dma_start(out=P["outv"][:], in_=res[:])

    nc.compile()
    return nc


# ============================================================================
# entry point
# ============================================================================

_CACHE = {}


def _get_program(debug_taps=()):
    key = (T, tuple(sorted(debug_taps)))
    if key not in _CACHE:
        _CACHE[key] = build_program(debug_taps=debug_taps)
    return _CACHE[key]


def kernel(emb, W_ih, W_hh, b_lstm, W_out, b_out, trans, start_t, end_t,
           tokens, tags, lengths, _debug_taps=(), _results_hook=None,
           _trace=False):
    emb = np.asarray(emb, dtype=np.float32)
    W_ih = np.asarray(W_ih, dtype=np.float32)
    W_hh = np.asarray(W_hh, dtype=np.float32)
    b_lstm = np.asarray(b_lstm, dtype=np.float32)
    W_out = np.asarray(W_out, dtype=np.float32)
    b_out = np.asarray(b_out, dtype=np.float32)
    trans = np.asarray(trans, dtype=np.float32)
    start_t = np.asarray(start_t, dtype=np.float32)
    end_t = np.asarray(end_t, dtype=np.float32)
    tokens = np.asarray(tokens)
    tags = np.asarray(tags)
    lengths = np.asarray(lengths)

    shared = prep_shared(emb, W_ih, W_hh, b_lstm, W_out, b_out, trans,
                         start_t, end_t)
    shared_io = {k: v for k, v in shared.items()}
    in_maps = []
    for core in range(NCORES):
        m = dict(shared_io)
        m.update(prep_core(core, tokens, tags, lengths, trans, start_t))
        in_maps.append(m)

    nc = _get_program(debug_taps=_debug_taps)
    res = run_bass_kernel_spmd(nc, in_maps, core_ids=list(range(NCORES)),
                               trace=_trace)
    if _results_hook is not None:
        _results_hook(res)
    total = np.float64(0.0)
    for core in range(NCORES):
        total += np.float64(res.results[core]["outv"][0, 0])
    return np.float32(total)



# revision 18
# speedup vs baseline: 1.1648x; 1.1648x over previous
"""BiLSTM-CRF forward NLL on 8 Trainium2 NeuronCores (Bass/Tile).

kernel(**inputs) takes the full unsharded inputs (as produced by the
reference setup_inputs) and returns the full output (scalar f32 NLL sum).

Sharding: data-parallel over batch (64 seqs -> 8 seqs/core); parameters
replicated. Per core:
  A. embedding gather (indirect DMA) + PE transpose -> xT [E_part, st] bf16
  B. xs = W_ih @ x + b as bf16 GEMMs (gate rows permuted to [i,f,o,g]);
     bwd xs overwritten with -40 past seq end (gates shut -> state stays 0,
     replacing per-step masking); xs staged to HBM
  C. LSTM scan, fwd+bwd as two interleaved chains; gates-on-partition
  D. layer-1 xs from hs0, second scan
  E. emissions GEMM straight into CRF tree layout
  F. CRF logZ via exp-space tree reduction over [8,8] transition matrices;
     gold score via host-built one-hot masks; out = sum(logZ - gold)
"""

import numpy as np
import ml_dtypes

import concourse.bacc as bacc
import concourse.bass as bass
import concourse.mybir as mybir
from concourse.tile import TileContext
from concourse.bass_utils import run_bass_kernel_spmd

F32 = mybir.dt.float32
BF16 = mybir.dt.bfloat16
I32 = mybir.dt.int32
F8 = mybir.dt.float8e3
NPF8 = ml_dtypes.float8_e3m4
AF = mybir.ActivationFunctionType
ALU = mybir.AluOpType

V, E, H, LBL, LAYERS = 50000, 512, 256, 8, 2
B = 64
NCORES = 8
BS = B // NCORES          # seqs per core
T = 1024                  # overridable for dev via build(T=...)
NBIG = -1.0e9
XPEN = -40.0
QP = 64                   # CRF matrices per partition

# gate order [f, i, g, o]: f/i in the lo sigmoid half (t2 starts early),
# g doubled for the sigmoid-tanh trick, o only needed late (h)
GPERM = np.concatenate([
    np.arange(256, 512), np.arange(0, 256),
    np.arange(512, 768), np.arange(768, 1024)])


# ============================================================================
# host-side preparation
# ============================================================================

def prep_shared(emb, W_ih, W_hh, b_lstm, W_out, b_out, trans, start_t, end_t):
    d = {}
    d["emb"] = np.ascontiguousarray(emb, dtype=np.float32)
    wih = np.empty((LAYERS, 2, 4, 8, 128, 128), dtype=np.float32)
    whh = np.empty((LAYERS, 2, 2, 8, 128, 128), dtype=np.float32)
    biases = np.empty((LAYERS, 2, 128, 8), dtype=np.float32)
    # g-gate rows (permuted order: gates 512:768 = m-chunks 4,5) are doubled
    # so a single sigmoid computes sigmoid(2x); tanh(x) = 2*sigmoid(2x)-1 via
    # a one-op DVE fixup.
    gdbl = np.ones((1024, 1), np.float32)
    gdbl[512:768] = 2.0
    for l in range(LAYERS):
        for dd in range(2):
            lt = (W_ih[l, dd][GPERM] * gdbl).T.astype(np.float32)  # [512,1024]
            wih[l, dd] = lt.reshape(4, 128, 8, 128).transpose(0, 2, 1, 3)
            lth = (W_hh[l, dd][GPERM] * gdbl).T.astype(np.float32)  # [256,1024]
            whh[l, dd] = lth.reshape(2, 128, 8, 128).transpose(0, 2, 1, 3)
            biases[l, dd] = (b_lstm[l, dd][GPERM] * gdbl[:, 0]).reshape(8, 128).T
    d["wih0"] = wih[0].astype(ml_dtypes.bfloat16)
    d["wih1"] = wih[1].astype(NPF8)
    d["whh"] = whh.astype(NPF8)
    d["biasv"] = biases
    d["woutT"] = np.ascontiguousarray(
        W_out.T.astype(np.float32).reshape(4, 128, 8)).astype(NPF8)
    d["bout_rep"] = np.broadcast_to(
        b_out.astype(np.float32)[None, :], (128, 8)).copy()
    d["identity"] = np.eye(128, dtype=np.float32)
    d["ntransflat"] = np.ascontiguousarray(
        -trans.astype(np.float32).reshape(64, 1))
    d["startv"] = start_t.astype(np.float32).reshape(8, 1).copy()
    d["endv"] = end_t.astype(np.float32).reshape(8, 1).copy()
    d["end_rep"] = np.broadcast_to(
        end_t.astype(np.float32)[None, :], (8, 8)).copy()
    d["ones8"] = np.ones((8, 1), dtype=np.float32)
    d["nones128"] = -np.ones((128, 1), dtype=np.float32)
    return d


def prep_core(core, tokens, tags, lengths, trans, start_t):
    ST = BS * T
    NP = ST // QP
    s0 = core * BS
    tok = tokens[s0:s0 + BS].astype(np.int64)
    tg = tags[s0:s0 + BS].astype(np.int64)
    ln = np.maximum(lengths[s0:s0 + BS].astype(np.int64), 1)

    st = np.arange(ST)
    st_u = st // T
    st_t = st % T

    d = {}
    d["tokidx"] = np.ascontiguousarray(
        tok[st_u, st_t].reshape(ST // 128, 128).T).astype(np.int32)

    mask_st = (st_t < ln[st_u]).astype(np.float32)
    d["xmask_rep"] = np.broadcast_to(
        mask_st[None, :], (128, ST)).astype(ml_dtypes.bfloat16)
    d["xpen_rep"] = np.broadcast_to(
        (XPEN * (1.0 - mask_st))[None, :], (128, ST)).astype(ml_dtypes.bfloat16)

    valid = mask_st > 0
    ohe = np.zeros((NP, QP * 8), dtype=np.float32)
    ohe[(st // QP)[valid], (st % QP)[valid] * 8 + tg[st_u[valid], st_t[valid]]] = 1.0
    d["ohe"] = ohe.astype(ml_dtypes.bfloat16)

    oh2 = np.zeros((64, ST), dtype=np.float32)
    v2 = (st_t >= 1) & (st_t < ln[st_u])
    pair = tg[st_u, st_t] * 8 + tg[st_u, np.maximum(st_t - 1, 0)]
    oh2[pair[v2], st[v2]] = 1.0
    d["oh2T"] = oh2.astype(ml_dtypes.bfloat16)

    cnt0 = np.zeros(8, dtype=np.float32)
    cntE = np.zeros(8, dtype=np.float32)
    for u in range(BS):
        cnt0[tg[u, 0]] += 1.0
        cntE[tg[u, ln[u] - 1]] += 1.0
    d["ncnt0"] = np.ascontiguousarray(-cnt0.reshape(8, 1))
    d["ncntE"] = np.ascontiguousarray(-cntE.reshape(8, 1))

    em = np.zeros((NP, QP), dtype=np.float32)
    em[st // QP, st % QP] = mask_st
    d["em"] = em

    trans32 = trans.astype(np.float32)
    start_mat = np.broadcast_to(start_t.astype(np.float32)[:, None], (8, 8))
    ilog = np.full((8, 8), NBIG, dtype=np.float32)
    np.fill_diagonal(ilog, 0.0)
    leaf = np.where(st_t[:, None, None] == 0, start_mat[None],
                    np.where((st_t < ln[st_u])[:, None, None], trans32[None],
                             ilog[None]))
    tmx = np.zeros((NP, QP * 64), dtype=np.float32)
    tmx[(st // QP)[:, None], ((st % QP) * 64)[:, None] + np.arange(64)[None, :]] = \
        leaf.reshape(ST, 64)
    d["tmx"] = tmx
    return d


# ============================================================================
# device program
# ============================================================================

def build_program(debug_taps=(), t_override=None):
    global T
    if t_override is not None:
        T = t_override
    ST = BS * T
    NP = ST // QP            # CRF partitions in use
    NC128 = ST // 128        # gather calls
    NSL = ST // 512          # 512-wide GEMM slices
    CH = min(64, T)          # scan prefetch chunk
    NCH = T // CH
    XLV = 6                  # in-partition tree levels (64 -> 1)
    CLV = int(np.log2(max(NP // BS, 1)))   # cross-partition levels

    nc = bacc.Bacc("TRN2", target_bir_lowering=False, debug=False,
                   num_devices=NCORES)
    dp = nc.declare_dram_parameter
    P = {}
    P["emb"] = dp("emb", [V, E], F32, isOutput=False)
    P["wih0"] = dp("wih0", [2, 4, 8, 128, 128], BF16, isOutput=False)
    P["wih1"] = dp("wih1", [2, 4, 8, 128, 128], F8, isOutput=False)
    P["whh"] = dp("whh", [LAYERS, 2, 2, 8, 128, 128], F8, isOutput=False)
    P["biasv"] = dp("biasv", [LAYERS, 2, 128, 8], F32, isOutput=False)
    P["woutT"] = dp("woutT", [4, 128, 8], F8, isOutput=False)
    P["bout_rep"] = dp("bout_rep", [128, 8], F32, isOutput=False)
    P["identity"] = dp("identity", [128, 128], F32, isOutput=False)
    P["ntransflat"] = dp("ntransflat", [64, 1], F32, isOutput=False)
    P["startv"] = dp("startv", [8, 1], F32, isOutput=False)
    P["endv"] = dp("endv", [8, 1], F32, isOutput=False)
    P["end_rep"] = dp("end_rep", [8, 8], F32, isOutput=False)
    P["ones8"] = dp("ones8", [8, 1], F32, isOutput=False)
    P["nones128"] = dp("nones128", [128, 1], F32, isOutput=False)
    P["tokidx"] = dp("tokidx", [128, NC128], I32, isOutput=False)
    P["xmask_rep"] = dp("xmask_rep", [128, ST], BF16, isOutput=False)
    P["xpen_rep"] = dp("xpen_rep", [128, ST], BF16, isOutput=False)
    P["ohe"] = dp("ohe", [NP, QP * 8], BF16, isOutput=False)
    P["oh2T"] = dp("oh2T", [64, ST], BF16, isOutput=False)
    P["ncnt0"] = dp("ncnt0", [8, 1], F32, isOutput=False)
    P["ncntE"] = dp("ncntE", [8, 1], F32, isOutput=False)
    P["em"] = dp("em", [NP, QP], F32, isOutput=False)
    P["tmx"] = dp("tmx", [NP, QP * 64], F32, isOutput=False)
    P["outv"] = dp("outv", [1, 1], F32, isOutput=True)
    xs_hbm = [nc.dram_tensor(f"xs_hbm{d}", [8, 128, ST], BF16)
              for d in range(2)]

    def dbg_dump(tc_nc, name, ap, shape, dtype):
        t = tc_nc.dram_tensor(name, shape, dtype, kind="ExternalOutput")
        tc_nc.sync.dma_start(out=t[:], in_=ap)

    with TileContext(nc) as tc:
        # ---- global constants ----
        with tc.tile_pool(name="gconst", bufs=1) as gc:
            wih_sb = {0: gc.tile([128, 2 * 4 * 8 * 128], BF16, tag="wih0", name="wih0sb"),
                      1: gc.tile([128, 2 * 4 * 8 * 128], F8, tag="wih1", name="wih1sb")}
            whh_sb = gc.tile([128, LAYERS * 2 * 2 * 8 * 128], F8, tag="whh")
            bias_sb = gc.tile([128, LAYERS * 2 * 8], F32, tag="bias")
            for l in range(LAYERS):
                for d in range(2):
                    o = d * 4 * 8 * 128
                    nc.sync.dma_start(
                        out=wih_sb[l][:, o:o + 4 * 8 * 128].rearrange(
                            "p (k m q) -> p k m q", k=4, m=8),
                        in_=P[f"wih{l}"][:][d].rearrange("k m p q -> p k m q"))
                    o = (l * 2 + d) * 2 * 8 * 128
                    nc.sync.dma_start(
                        out=whh_sb[:, o:o + 2 * 8 * 128].rearrange(
                            "p (k m q) -> p k m q", k=2, m=8),
                        in_=P["whh"][:][l, d].rearrange("k m p q -> p k m q"))
                    o = (l * 2 + d) * 8
                    nc.sync.dma_start(out=bias_sb[:, o:o + 8],
                                      in_=P["biasv"][:][l, d])
            ident_sb = gc.tile([128, 128], F32, tag="ident")
            nc.sync.dma_start(out=ident_sb[:], in_=P["identity"][:])
            identb_sb = gc.tile([128, 128], BF16, tag="identb")
            nc.vector.tensor_copy(out=identb_sb[:], in_=ident_sb[:])
            xmask_sb = gc.tile([128, ST], BF16, tag="xmask")
            nc.sync.dma_start(out=xmask_sb[:], in_=P["xmask_rep"][:])
            xpen_sb = gc.tile([128, ST], BF16, tag="xpen")
            nc.sync.dma_start(out=xpen_sb[:], in_=P["xpen_rep"][:])

            def wih_t(l, d, kc, m):
                i = (d * 4 + kc) * 8 + m
                return wih_sb[l][:, i * 128:(i + 1) * 128]

            def whh_t(l, d, kc, m):
                i = ((l * 2 + d) * 2 + kc) * 8 + m
                return whh_sb[:, i * 128:(i + 1) * 128]

            def bias_col(l, d, m):
                i = (l * 2 + d) * 8 + m
                return bias_sb[:, i:i + 1]

            # ---- input-projection GEMM (shared by both layers) ----
            def xproj_slice(l, rhs_chunk, sp, pp, d, m, s):
                ps = pp.tile([128, 512], F32, tag="g")
                for kc in range(4):
                    nc.tensor.matmul(
                        ps[:], lhsT=wih_t(l, d, kc, m),
                        rhs=rhs_chunk(kc)[:, s * 512:(s + 1) * 512],
                        start=(kc == 0), stop=(kc == 3))
                stg = sp.tile([128, 512], BF16, tag="xstage")
                if d == 0:
                    nc.vector.tensor_scalar_add(
                        stg[:], ps[:], bias_col(l, d, m))
                else:
                    nc.vector.scalar_tensor_tensor(
                        out=stg[:], in0=ps[:],
                        scalar=bias_col(l, d, m),
                        in1=xmask_sb[:, s * 512:(s + 1) * 512],
                        op0=ALU.add, op1=ALU.mult)
                    nc.vector.tensor_tensor(
                        out=stg[:], in0=stg[:],
                        in1=xpen_sb[:, s * 512:(s + 1) * 512],
                        op=ALU.add)
                nc.sync.dma_start(
                    out=xs_hbm[d][:][m, :, s * 512:(s + 1) * 512],
                    in_=stg[:])

            def xproj_layer(l, rhs_chunk):
                with tc.tile_pool(name=f"xp{l}", bufs=4) as sp, \
                     tc.tile_pool(name=f"xpp{l}", bufs=2, space="PSUM") as pp:
                    for d in range(2):
                        for m in range(8):
                            for s in range(NSL):
                                xproj_slice(l, rhs_chunk, sp, pp, d, m, s)

            # ---- LSTM scan (both dirs interleaved) ----
            def scan_layer(l, hs):
                with tc.tile_pool(name=f"sc{l}", bufs=4) as sp, \
                     tc.tile_pool(name=f"scs{l}", bufs=2) as strm, \
                     tc.tile_pool(name=f"scst{l}", bufs=1) as stp, \
                     tc.tile_pool(name=f"scp{l}", bufs=2, space="PSUM") as pp:
                    c_st = {d: stp.tile([128, 16], F32, tag=f"c{d}", name=f"c{d}")
                            for d in range(2)}
                    for d in range(2):
                        nc.vector.memset(c_st[d][:], 0.0)

                    def fetch(d, k):
                        buf = strm.tile([128, 64 * CH], BF16, tag=f"xsb{d}", name=f"xsb{d}")
                        t0 = k * CH
                        for m in range(8):
                            nc.sync.dma_start(
                                out=buf[:, m * BS * CH:(m + 1) * BS * CH]
                                .rearrange("p (u ch) -> p u ch", u=BS),
                                in_=xs_hbm[d][:][m].rearrange(
                                    "p (u t) -> p u t", u=BS)[:, :, t0:t0 + CH])
                        return buf

                    def id_phase(d, t, buf):
                        # xs injection: identity matmuls run while PE waits
                        # for h. lo/hi are separate PSUM tiles = separate
                        # accumulation groups, each with own start and stop.
                        trel = t % CH
                        xs_t = buf[:, trel::CH]
                        first = (t == 0) if d == 0 else (t == T - 1)
                        Glo = pp.tile([128, 32], F32, tag=f"Glo{d}")
                        Ghi = pp.tile([128, 32], F32, tag=f"Ghi{d}")
                        nc.tensor.matmul(Glo[:], lhsT=identb_sb[:],
                                         rhs=xs_t[:, 0:32], start=True,
                                         stop=first)
                        nc.tensor.matmul(Ghi[:], lhsT=identb_sb[:],
                                         rhs=xs_t[:, 32:64], start=True,
                                         stop=first)
                        return Glo, Ghi

                    def whh_phase(d, t, G2):
                        # gate order [f,i | g,o]: lo half finishes first so
                        # sigmoid-lo/t2 start before the hi MMs end; kc-major
                        # within each half so the burst head waits h-chunk0
                        # only
                        Glo, Ghi = G2
                        first = (t == 0) if d == 0 else (t == T - 1)
                        if first:
                            return
                        tprev = t - 1 if d == 0 else t + 1
                        hv = hs[d][:].rearrange(
                            "p (c u t) -> p c u t", c=2, u=BS)
                        for g0, Gt in ((0, Glo), (4, Ghi)):
                            for kc in range(2):
                                for m in range(g0, g0 + 4):
                                    mm = m - g0
                                    nc.tensor.matmul(
                                        Gt[:, mm * 8:(mm + 1) * 8],
                                        lhsT=whh_t(l, d, kc, m),
                                        rhs=hv[:, kc, :, tprev],
                                        start=False,
                                        stop=(kc == 1 and m == g0 + 3))

                    def sig_lo(d, Glo):
                        S = sp.tile([128, 64], BF16, tag=f"S{d}")
                        nc.scalar.activation(S[:, 0:32], Glo[:], AF.Sigmoid)
                        return S

                    def sig_hi(d, Ghi, S):
                        nc.scalar.activation(S[:, 32:64], Ghi[:], AF.Sigmoid)

                    def t2_phase(d, S):
                        # t2 = sig(f) * c_prev ; f-gates live in the lo half
                        t2 = sp.tile([128, 16], F32, tag=f"t2{d}")
                        nc.vector.tensor_tensor(out=t2[:], in0=S[:, 0:16],
                                                in1=c_st[d][:], op=ALU.mult)
                        return t2

                    def c_phase(d, S, t2):
                        # t1h = (sig(2g) - 0.5) * i  == 0.5 * i * tanh(g)
                        t1 = sp.tile([128, 16], F32, tag=f"t1{d}")
                        nc.vector.scalar_tensor_tensor(
                            out=t1[:], in0=S[:, 32:48], scalar=0.5,
                            in1=S[:, 16:32], op0=ALU.subtract, op1=ALU.mult)
                        # c = 2*t1h + t2
                        nc.vector.scalar_tensor_tensor(
                            out=c_st[d][:], in0=t1[:], scalar=2.0, in1=t2[:],
                            op0=ALU.mult, op1=ALU.add)

                    def tanh_phase(d):
                        Tc = sp.tile([128, 16], F32, tag=f"Tc{d}")
                        nc.scalar.activation(Tc[:], c_st[d][:], AF.Tanh)
                        return Tc

                    def h_phase(d, t, S, Tc):
                        # split by h-chunk so next step's kc0 matmuls gate on
                        # chunk0 only
                        hv = hs[d][:].rearrange(
                            "p (c u t) -> p c u t", c=2, u=BS)
                        nc.vector.tensor_tensor(out=hv[:, 0, :, t],
                                                in0=S[:, 48:56],
                                                in1=Tc[:, 0:8], op=ALU.mult)
                        nc.vector.tensor_tensor(out=hv[:, 1, :, t],
                                                in0=S[:, 56:64],
                                                in1=Tc[:, 8:16], op=ALU.mult)

                    bufs = {0: fetch(0, 0), 1: fetch(1, NCH - 1)}
                    for k in range(NCH):
                        nxt = None
                        if k + 1 < NCH:
                            nxt = (fetch(0, k + 1), fetch(1, NCH - 2 - k))
                        for i in range(CH):
                            tf = k * CH + i
                            tb = T - 1 - tf
                            # phase-staggered issue: each engine's FIFO sees
                            # the two chains' ops in dependency-friendly order
                            Gf = id_phase(0, tf, bufs[0])
                            whh_phase(0, tf, Gf)
                            Sf = sig_lo(0, Gf[0])
                            Gb = id_phase(1, tb, bufs[1])
                            whh_phase(1, tb, Gb)
                            t2f = t2_phase(0, Sf)
                            sig_hi(0, Gf[1], Sf)
                            Sb = sig_lo(1, Gb[0])
                            c_phase(0, Sf, t2f)
                            sig_hi(1, Gb[1], Sb)
                            t2b = t2_phase(1, Sb)
                            Tcf = tanh_phase(0)
                            c_phase(1, Sb, t2b)
                            Tcb = tanh_phase(1)
                            h_phase(0, tf, Sf, Tcf)
                            h_phase(1, tb, Sb, Tcb)
                        if nxt is not None:
                            bufs[0], bufs[1] = nxt

            # ================= pipeline =================
            with tc.tile_pool(name="xt", bufs=1) as xt_pool:
                xT = xt_pool.tile([128, 4 * ST], BF16, tag="xT")
                rhs0 = lambda kc: xT[:, kc * ST:(kc + 1) * ST]
                with tc.tile_pool(name="gat", bufs=4) as gp, \
                     tc.tile_pool(name="gatp", bufs=4, space="PSUM") as gpp, \
                     tc.tile_pool(name="tokp", bufs=1) as tkp, \
                     tc.tile_pool(name="xp0", bufs=4) as xsp, \
                     tc.tile_pool(name="xpp0", bufs=2, space="PSUM") as xpp:
                    tok_sb = tkp.tile([128, NC128], I32, tag="tok")
                    nc.sync.dma_start(out=tok_sb[:], in_=P["tokidx"][:])
                    for j in range(NC128):
                        g = gp.tile([128, E], F32, tag="xg")
                        nc.gpsimd.indirect_dma_start(
                            out=g[:], out_offset=None, in_=P["emb"][:],
                            in_offset=bass.IndirectOffsetOnAxis(
                                ap=tok_sb[:, j:j + 1], axis=0))
                        for c in range(4):
                            pst = gpp.tile([128, 128], F32, tag="tp")
                            nc.tensor.transpose(
                                out=pst[:], in_=g[:, c * 128:(c + 1) * 128],
                                identity=ident_sb[:])
                            nc.vector.tensor_copy(
                                out=xT[:, c * ST + j * 128:c * ST + (j + 1) * 128],
                                in_=pst[:])
                        # xproj0 slice s only needs xT column-blocks <= j,
                        # so overlap the layer-0 GEMM with the gather
                        if j % 4 == 3:
                            s = j // 4
                            for d in range(2):
                                for m in range(8):
                                    xproj_slice(0, rhs0, xsp, xpp, d, m, s)
                if "xT" in debug_taps:
                    dbg_dump(nc, "dbg_xT", xT[:], [128, 4 * ST], BF16)

            with tc.tile_pool(name="hs0", bufs=1) as hs0_pool:
                hs0 = {d: hs0_pool.tile([128, 2 * ST], F8, tag=f"h{d}", name=f"hs0{d}")
                       for d in range(2)}
                scan_layer(0, hs0)
                if "hs0" in debug_taps:
                    for d in range(2):
                        dbg_dump(nc, f"dbg_hs0{d}", hs0[d][:], [128, 2 * ST], F8)
                xproj_layer(1, lambda kc: hs0[kc // 2][
                    :, (kc % 2) * ST:(kc % 2 + 1) * ST])

            with tc.tile_pool(name="earr", bufs=1) as ea_pool:
                e_arr = ea_pool.tile([NP, QP * 8], F32, tag="e_arr")
                with tc.tile_pool(name="hs1", bufs=1) as hs1_pool:
                    hs1 = {d: hs1_pool.tile([128, 2 * ST], F8, tag=f"h{d}", name=f"hs1{d}")
                           for d in range(2)}
                    scan_layer(1, hs1)
                    if "hs1" in debug_taps:
                        for d in range(2):
                            dbg_dump(nc, f"dbg_hs1{d}", hs1[d][:],
                                     [128, 2 * ST], F8)
                    # emissions into CRF layout
                    with tc.tile_pool(name="emc", bufs=1) as emc, \
                         tc.tile_pool(name="emp", bufs=2, space="PSUM") as epp:
                        woutT_sb = emc.tile([128, 4 * 8], F8, tag="wout")
                        nc.sync.dma_start(
                            out=woutT_sb[:].rearrange("p (k m) -> p k m", k=4),
                            in_=P["woutT"][:].rearrange("k p m -> p k m"))
                        bout_sb = emc.tile([128, 8], F32, tag="bout")
                        nc.sync.dma_start(out=bout_sb[:], in_=P["bout_rep"][:])
                        for q in range(QP):
                            ps = epp.tile([NP, 8], F32, tag="eps")
                            for kc in range(4):
                                d, c = divmod(kc, 2)
                                lh = hs1[d][:].rearrange(
                                    "p (c pp q) -> p c pp q", c=2, q=QP)[
                                    :, c, :, q]
                                nc.tensor.matmul(
                                    ps[:], lhsT=lh,
                                    rhs=woutT_sb[:, kc * 8:(kc + 1) * 8],
                                    start=(kc == 0), stop=(kc == 3))
                            nc.vector.tensor_tensor(
                                out=e_arr[:, q * 8:(q + 1) * 8], in0=ps[:],
                                in1=bout_sb[:NP, :], op=ALU.add)
                if "e_arr" in debug_taps:
                    dbg_dump(nc, "dbg_e_arr", e_arr[:], [NP, QP * 8], F32)

                # ---------------- CRF ----------------
                with tc.tile_pool(name="crfc", bufs=1) as cc, \
                     tc.tile_pool(name="crfw", bufs=1) as cw, \
                     tc.tile_pool(name="crfp", bufs=1, space="PSUM") as cpp:
                    em_sb = cc.tile([NP, QP], F32, tag="em")
                    nc.sync.dma_start(out=em_sb[:], in_=P["em"][:])
                    tmx_sb = cc.tile([NP, QP * 64], F32, tag="tmx")
                    nc.sync.dma_start(out=tmx_sb[:], in_=P["tmx"][:])
                    ohe_sb = cc.tile([NP, QP * 8], BF16, tag="ohe")
                    nc.sync.dma_start(out=ohe_sb[:], in_=P["ohe"][:])
                    oh2_sb = cc.tile([64, ST], BF16, tag="oh2")
                    nc.sync.dma_start(out=oh2_sb[:], in_=P["oh2T"][:])
                    ntr_sb = cc.tile([64, 1], F32, tag="ntr")
                    nc.sync.dma_start(out=ntr_sb[:], in_=P["ntransflat"][:])
                    sv_sb = cc.tile([8, 1], F32, tag="sv")
                    nc.sync.dma_start(out=sv_sb[:], in_=P["startv"][:])
                    ev_sb = cc.tile([8, 1], F32, tag="ev")
                    nc.sync.dma_start(out=ev_sb[:], in_=P["endv"][:])
                    erep_sb = cc.tile([8, 8], F32, tag="erep")
                    nc.sync.dma_start(out=erep_sb[:], in_=P["end_rep"][:])
                    on8_sb = cc.tile([8, 1], F32, tag="on8")
                    nc.sync.dma_start(out=on8_sb[:], in_=P["ones8"][:])
                    no128_sb = cc.tile([128, 1], F32, tag="no128")
                    nc.sync.dma_start(out=no128_sb[:], in_=P["nones128"][:])
                    nc0_sb = cc.tile([8, 1], F32, tag="nc0")
                    nc.sync.dma_start(out=nc0_sb[:], in_=P["ncnt0"][:])
                    ncE_sb = cc.tile([8, 1], F32, tag="ncE")
                    nc.sync.dma_start(out=ncE_sb[:], in_=P["ncntE"][:])

                    # leaves: M = e*em + tmx  (log space), then exp w/ max
                    X = cw.tile([NP, QP * 64], F32, tag="X")
                    e_b = e_arr[:].rearrange("p (q i) -> p q i", i=8) \
                        .unsqueeze(3).to_broadcast([NP, QP, 8, 8])
                    em_b = em_sb[:].unsqueeze(2).unsqueeze(3) \
                        .to_broadcast([NP, QP, 8, 8])
                    Xv = X[:].rearrange("p (q i j) -> p q i j", i=8, j=8)
                    nc.vector.tensor_tensor(out=Xv, in0=e_b, in1=em_b,
                                            op=ALU.mult)
                    nc.vector.tensor_tensor(out=X[:], in0=X[:], in1=tmx_sb[:],
                                            op=ALU.add)
                    Sc = cw.tile([NP, QP], F32, tag="Sc")
                    nc.vector.tensor_reduce(
                        out=Sc[:].unsqueeze(2),
                        in_=X[:].rearrange("p (q e) -> p q e", e=64),
                        axis=mybir.AxisListType.X, op=ALU.max)
                    scb = Sc[:].unsqueeze(2).to_broadcast([NP, QP, 64]) \
                        .rearrange("p q e -> p q e")
                    nc.vector.tensor_tensor(
                        out=X[:].rearrange("p (q e) -> p q e", e=64),
                        in0=X[:].rearrange("p (q e) -> p q e", e=64),
                        in1=scb, op=ALU.subtract)
                    nc.scalar.activation(X[:], X[:], AF.Exp)

                    tmpa = cw.tile([NP, (QP // 2) * 64], F32, tag="tmpa")
                    tmpb = cw.tile([NP, (QP // 2) * 64], F32, tag="tmpb")

                    def combine(Xa, Xb, Sa, Sb, Pn, r, Y, Sy):
                        """Y = Xa x Xb (exp-space matmul over k), renormed.
                        Xa/Xb views [Pn, r, 8, 8]; Sa/Sb [Pn, r]; Y [Pn, r*64]."""
                        Yv = Y[:Pn, :r * 64].rearrange(
                            "p (r i j) -> p r i j", i=8, j=8)
                        Tv = tmpa[:Pn, :r * 64].rearrange(
                            "p (r i j) -> p r i j", i=8, j=8)
                        for kk in range(8):
                            ak = Xa[:, :, :, kk].unsqueeze(3) \
                                .to_broadcast([Pn, r, 8, 8])
                            bk = Xb[:, :, kk, :].unsqueeze(2) \
                                .to_broadcast([Pn, r, 8, 8])
                            dst = Yv if kk == 0 else Tv
                            nc.vector.tensor_tensor(out=dst, in0=ak, in1=bk,
                                                    op=ALU.mult)
                            if kk > 0:
                                nc.vector.tensor_tensor(out=Yv, in0=Yv, in1=Tv,
                                                        op=ALU.add)
                        # renormalize
                        am = tmpb[:Pn, :r]
                        nc.vector.tensor_reduce(
                            out=am.unsqueeze(2),
                            in_=Y[:Pn, :r * 64].rearrange("p (r e) -> p r e",
                                                          e=64),
                            axis=mybir.AxisListType.X, op=ALU.max)
                        inv = tmpb[:Pn, r:2 * r]
                        nc.vector.reciprocal(out=inv, in_=am)
                        nc.vector.tensor_tensor(
                            out=Y[:Pn, :r * 64].rearrange("p (r e) -> p r e", e=64),
                            in0=Y[:Pn, :r * 64].rearrange("p (r e) -> p r e", e=64),
                            in1=inv.unsqueeze(2).to_broadcast([Pn, r, 64]),
                            op=ALU.mult)
                        lam = tmpb[:Pn, 2 * r:3 * r]
                        nc.scalar.activation(lam, am, AF.Ln)
                        nc.vector.tensor_tensor(out=Sy, in0=Sa, in1=Sb,
                                                op=ALU.add)
                        nc.vector.tensor_tensor(out=Sy, in0=Sy, in1=lam,
                                                op=ALU.add)

                    # in-partition levels
                    Y = cw.tile([NP, (QP // 2) * 64], F32, tag="Y")
                    S2 = cw.tile([NP, QP], F32, tag="S2")
                    curX, curS = X, Sc
                    dstX, dstS = Y, S2
                    n = QP
                    while n > 1:
                        r = n // 2
                        Xq = curX[:NP, :n * 64].rearrange(
                            "p (r two i j) -> p r two i j", two=2, i=8, j=8)
                        combine(Xq[:, :, 1], Xq[:, :, 0],
                                curS[:NP, 1:n:2], curS[:NP, 0:n:2],
                                NP, r, dstX, dstS[:NP, :r])
                        curX, dstX = dstX, curX
                        curS, dstS = dstS, curS
                        n = r
                    # cross-partition levels: Pn partitions -> Pn/2
                    Pn = NP
                    Za = cw.tile([max(NP // 2, 8), 64], F32, tag="Za")
                    Zb = cw.tile([max(NP // 2, 8), 64], F32, tag="Zb")
                    Zsa = cw.tile([max(NP // 2, 8), 1], F32, tag="Zsa")
                    Zsb = cw.tile([max(NP // 2, 8), 1], F32, tag="Zsb")
                    for _ in range(CLV):
                        h = Pn // 2
                        nc.sync.dma_start(out=Za[:h, :], in_=curX[1:Pn:2, :64])
                        nc.sync.dma_start(out=Zb[:h, :], in_=curX[0:Pn:2, :64])
                        nc.sync.dma_start(out=Zsa[:h, :], in_=curS[1:Pn:2, 0:1])
                        nc.sync.dma_start(out=Zsb[:h, :], in_=curS[0:Pn:2, 0:1])
                        combine(Za[:h, :].rearrange("p (o i j) -> p o i j",
                                                    o=1, i=8, j=8),
                                Zb[:h, :].rearrange("p (o i j) -> p o i j",
                                                    o=1, i=8, j=8),
                                Zsa[:h, :], Zsb[:h, :],
                                h, 1, dstX, dstS[:h, :1])
                        curX, dstX = dstX, curX
                        curS, dstS = dstS, curS
                        Pn = h
                    # root: curX [BS, 64] exp-space, curS [BS, 1]
                    eend = cw.tile([8, 8], F32, tag="eend")
                    nc.scalar.activation(eend[:], erep_sb[:], AF.Exp)
                    w = cw.tile([8, 8], F32, tag="w")
                    nc.vector.tensor_tensor(out=w[:], in0=curX[:BS, 0:64:8],
                                            in1=eend[:], op=ALU.mult)
                    zs = cw.tile([8, 1], F32, tag="zs")
                    nc.vector.tensor_reduce(out=zs[:], in_=w[:],
                                            axis=mybir.AxisListType.X,
                                            op=ALU.add)
                    nc.scalar.activation(zs[:], zs[:], AF.Ln)
                    logz = cw.tile([8, 1], F32, tag="logz")
                    nc.vector.tensor_tensor(out=logz[:], in0=zs[:],
                                            in1=curS[:BS, 0:1], op=ALU.add)
                    if "logz" in debug_taps:
                        dbg_dump(nc, "dbg_logz", logz[:], [8, 1], F32)

                    # ---- gold + final sum into one psum scalar ----
                    acc = cpp.tile([1, 1], F32, tag="acc")
                    nc.tensor.matmul(acc[:], lhsT=on8_sb[:], rhs=logz[:],
                                     start=True, stop=False)
                    # emission gold term (negated)
                    prod = cw.tile([NP, QP * 8], F32, tag="prod")
                    nc.vector.tensor_tensor(out=prod[:], in0=e_arr[:],
                                            in1=ohe_sb[:], op=ALU.mult)
                    red = cw.tile([NP, 1], F32, tag="red")
                    nc.vector.tensor_reduce(out=red[:], in_=prod[:],
                                            axis=mybir.AxisListType.X,
                                            op=ALU.add)
                    nc.tensor.matmul(acc[:], lhsT=no128_sb[:NP, :], rhs=red[:],
                                     start=False, stop=False)
                    # transition gold term (negated)
                    rs2 = cw.tile([64, 1], F32, tag="rs2")
                    nc.vector.tensor_reduce(out=rs2[:], in_=oh2_sb[:],
                                            axis=mybir.AxisListType.X,
                                            op=ALU.add)
                    nc.tensor.matmul(acc[:], lhsT=ntr_sb[:], rhs=rs2[:],
                                     start=False, stop=False)
                    # start/end gold terms (negated counts)
                    nc.tensor.matmul(acc[:], lhsT=nc0_sb[:], rhs=sv_sb[:],
                                     start=False, stop=False)
                    nc.tensor.matmul(acc[:], lhsT=ncE_sb[:], rhs=ev_sb[:],
                                     start=False, stop=True)
                    res = cw.tile([1, 1], F32, tag="res")
                    nc.vector.tensor_copy(out=res[:], in_=acc[:])
                    nc.sync.dma_start(out=P["outv"][:], in_=res[:])

    nc.compile()
    return nc


# ============================================================================
# entry point
# ============================================================================

_CACHE = {}


def _get_program(debug_taps=()):
    key = (T, tuple(sorted(debug_taps)))
    if key not in _CACHE:
        _CACHE[key] = build_program(debug_taps=debug_taps)
    return _CACHE[key]


def kernel(emb, W_ih, W_hh, b_lstm, W_out, b_out, trans, start_t, end_t,
           tokens, tags, lengths, _debug_taps=(), _results_hook=None,
           _trace=False):
    emb = np.asarray(emb, dtype=np.float32)
    W_ih = np.asarray(W_ih, dtype=np.float32)
    W_hh = np.asarray(W_hh, dtype=np.float32)
    b_lstm = np.asarray(b_lstm, dtype=np.float32)
    W_out = np.asarray(W_out, dtype=np.float32)
    b_out = np.asarray(b_out, dtype=np.float32)
    trans = np.asarray(trans, dtype=np.float32)
    start_t = np.asarray(start_t, dtype=np.float32)
    end_t = np.asarray(end_t, dtype=np.float32)
    tokens = np.asarray(tokens)
    tags = np.asarray(tags)
    lengths = np.asarray(lengths)

    shared = prep_shared(emb, W_ih, W_hh, b_lstm, W_out, b_out, trans,
                         start_t, end_t)
    shared_io = {k: v for k, v in shared.items()}
    in_maps = []
    for core in range(NCORES):
        m = dict(shared_io)
        m.update(prep_core(core, tokens, tags, lengths, trans, start_t))
        in_maps.append(m)

    nc = _get_program(debug_taps=_debug_taps)
    res = run_bass_kernel_spmd(nc, in_maps, core_ids=list(range(NCORES)),
                               trace=_trace)
    if _results_hook is not None:
        _results_hook(res)
    total = np.float64(0.0)
    for core in range(NCORES):
        total += np.float64(res.results[core]["outv"][0, 0])
    return np.float32(total)

